# revision 1
# baseline (speedup 1.0000x reference)
"""Squeeze-Excitation attention block on 8 TRN2 NeuronCores.

out = x * sigmoid(w2 @ relu(w1 @ mean(x, spatial) + b1) + b2)
x: [32, 256, 112, 112] f32.

Sharding: data-parallel over batch — 4 samples per core, weights replicated.
Per-core kernel streams x twice (pass 1: global-avg-pool reduce, pass 2:
gate multiply) with full-row [128, 12544] tiles so each DMA is 6.4MB.
"""
import numpy as np
from contextlib import ExitStack

import concourse.bass as bass
import concourse.tile as tile
from concourse import bacc, mybir
from concourse.bass_utils import run_bass_kernel_spmd

N_CORES = 8
B_PER_CORE = 4
C = 256
CR = 64
HALF = 2  # channel halves of 128
S = 112 * 112  # 12544
P = 128

F32 = mybir.dt.float32
AF = mybir.ActivationFunctionType
AX = mybir.AxisListType


def emit_body(tc, aps, pools):
    """Emit one full SE-block pass over the per-core shard."""
    nc = tc.nc
    x_ap, out_ap, w1t_ap, b1c_ap, w2t_ap, b2c_ap = aps
    consts, xs, psum, small = pools

    # --- constants (loaded once; emit_body may be called repeatedly, the
    # consts tiles are allocated by the first call and passed back in) ---
    if "w1_sb" not in consts:
        w1_sb = consts["pool"].tile([P, 2 * CR], F32, tag="w1_sb")
        nc.gpsimd.dma_start(w1_sb[:, 0:CR], w1t_ap[0])
        nc.gpsimd.dma_start(w1_sb[:, CR : 2 * CR], w1t_ap[1])
        w2_sb = consts["pool"].tile([CR, C], F32, tag="w2_sb")
        nc.gpsimd.dma_start(w2_sb[:], w2t_ap[:])
        b1_sb = consts["pool"].tile([CR, 1], F32, tag="b1_sb")
        nc.gpsimd.dma_start(b1_sb[:], b1c_ap[:])
        b2_sb = consts["pool"].tile([P, HALF], F32, tag="b2_sb")
        nc.gpsimd.dma_start(b2_sb[:], b2c_ap[:])
        consts.update(w1_sb=w1_sb, w2_sb=w2_sb, b1_sb=b1_sb, b2_sb=b2_sb)
    w1_sb = consts["w1_sb"]
    w2_sb = consts["w2_sb"]
    b1_sb = consts["b1_sb"]
    b2_sb = consts["b2_sb"]

    gap = small.tile([P, B_PER_CORE * HALF], F32, tag="gap")
    gate = small.tile([P, B_PER_CORE * HALF], F32, tag="gate")

    for b in range(B_PER_CORE):
        # pass 1: load both channel-halves, reduce over spatial
        for h in range(HALF):
            j = b * HALF + h
            t = xs.tile([P, S], F32, tag="xtile")
            nc.sync.dma_start(t[:], x_ap[b, h * P : (h + 1) * P, :])
            nc.vector.reduce_sum(gap[:, j : j + 1], t[:], axis=AX.X)

        # tiny MLP for this sample: gate = sigmoid(w2 @ relu(w1t' @ gapsum + b1) + b2)
        # (1/S mean scale is folded into w1t on the host)
        hp = psum.tile([CR, 1], F32, tag="hp")
        nc.tensor.matmul(
            hp[:], w1_sb[:, 0:CR], gap[:, b * HALF : b * HALF + 1],
            start=True, stop=False,
        )
        nc.tensor.matmul(
            hp[:], w1_sb[:, CR : 2 * CR], gap[:, b * HALF + 1 : b * HALF + 2],
            start=False, stop=True,
        )
        h_sb = small.tile([CR, 1], F32, tag="h_sb")
        nc.scalar.activation(h_sb[:], hp[:], AF.Relu, bias=b1_sb[:, 0:1])
        for h in range(HALF):
            j = b * HALF + h
            apv = psum.tile([P, 1], F32, tag="apv")
            nc.tensor.matmul(
                apv[:], w2_sb[:, h * P : (h + 1) * P], h_sb[:],
                start=True, stop=True,
            )
            nc.scalar.activation(
                gate[:, j : j + 1], apv[:], AF.Sigmoid, bias=b2_sb[:, h : h + 1]
            )

        # pass 2: reload, multiply by per-channel gate, store
        for h in range(HALF):
            j = b * HALF + h
            t2 = xs.tile([P, S], F32, tag="xtile")
            nc.sync.dma_start(t2[:], x_ap[b, h * P : (h + 1) * P, :])
            nc.vector.tensor_scalar_mul(t2[:], t2[:], gate[:, j : j + 1])
            nc.scalar.dma_start(out_ap[b, h * P : (h + 1) * P, :], t2[:])


def build_program(repeats=1):
    nc = bacc.Bacc("TRN2", target_bir_lowering=False, debug=False, num_devices=N_CORES)
    x_ap = nc.dram_tensor("x", [B_PER_CORE, C, S], F32, kind="ExternalInput").ap()
    w1t_ap = nc.dram_tensor("w1t", [2, P, CR], F32, kind="ExternalInput").ap()
    b1c_ap = nc.dram_tensor("b1c", [CR, 1], F32, kind="ExternalInput").ap()
    w2t_ap = nc.dram_tensor("w2t", [CR, C], F32, kind="ExternalInput").ap()
    b2c_ap = nc.dram_tensor("b2c", [P, HALF], F32, kind="ExternalInput").ap()
    out_ap = nc.dram_tensor("out", [B_PER_CORE, C, S], F32, kind="ExternalOutput").ap()
    aps = (x_ap, out_ap, w1t_ap, b1c_ap, w2t_ap, b2c_ap)

    with tile.TileContext(nc) as tc:
        with ExitStack() as ctx:
            consts_pool = ctx.enter_context(tc.tile_pool(name="consts", bufs=1))
            xs = ctx.enter_context(tc.tile_pool(name="xs", bufs=3))
            psum = ctx.enter_context(tc.tile_pool(name="psum", bufs=2, space="PSUM"))
            small = ctx.enter_context(tc.tile_pool(name="small", bufs=2))
            consts = {"pool": consts_pool}
            pools = (consts, xs, psum, small)
            for _ in range(repeats):
                emit_body(tc, aps, pools)
    nc.compile()
    return nc


def prep_inputs(x, w1, b1, w2, b2):
    """Host-side input prep: shard x by batch, fold mean-scale into w1."""
    xs = np.ascontiguousarray(x.reshape(32, C, S))
    w1t = np.ascontiguousarray((w1.T / S).astype(np.float32).reshape(2, P, CR))
    b1c = np.ascontiguousarray(b1.reshape(CR, 1).astype(np.float32))
    w2t = np.ascontiguousarray(w2.T.astype(np.float32))
    b2c = np.ascontiguousarray(b2.reshape(HALF, P).T.astype(np.float32))
    in_maps = []
    for c in range(N_CORES):
        in_maps.append(
            {
                "x": np.ascontiguousarray(xs[c * B_PER_CORE : (c + 1) * B_PER_CORE]),
                "w1t": w1t,
                "b1c": b1c,
                "w2t": w2t,
                "b2c": b2c,
            }
        )
    return in_maps


def kernel(x, w1, b1, w2, b2):
    x = np.asarray(x, dtype=np.float32)
    in_maps = prep_inputs(
        x, np.asarray(w1), np.asarray(b1), np.asarray(w2), np.asarray(b2)
    )
    nc = build_program()
    res = run_bass_kernel_spmd(nc, in_maps, list(range(N_CORES))).results
    out = np.concatenate([res[c]["out"] for c in range(N_CORES)], axis=0)
    return out.reshape(32, C, 112, 112)


# revision 4
# speedup vs baseline: 12.3641x; 12.3641x over previous
"""Squeeze-Excitation attention block on 8 TRN2 NeuronCores.

out = x * sigmoid(w2 @ relu(w1 @ mean(x, spatial) + b1) + b2)
x: [32, 256, 112, 112] f32.

Sharding: data-parallel over batch — 4 samples per core, weights replicated.
Per-core kernel streams x twice (pass 1: global-avg-pool reduce, pass 2:
gate multiply) with full-row [128, 12544] tiles so each DMA is 6.4MB.
"""
import numpy as np
from contextlib import ExitStack

import concourse.bass as bass
import concourse.tile as tile
from concourse import bacc, mybir
from concourse.bass_utils import run_bass_kernel_spmd

N_CORES = 8
B_PER_CORE = 4
C = 256
CR = 64
HALF = 2  # channel halves of 128
S = 112 * 112  # 12544
P = 128

F32 = mybir.dt.float32
AF = mybir.ActivationFunctionType
AX = mybir.AxisListType


def emit_body(tc, aps, pools):
    """Emit one full SE-block pass over the per-core shard."""
    nc = tc.nc
    x_ap, out_ap, w1t_ap, b1c_ap, w2t_ap, b2c_ap = aps
    consts, xs, psum, small = pools

    # --- constants (loaded once; emit_body may be called repeatedly, the
    # consts tiles are allocated by the first call and passed back in) ---
    if "w1_sb" not in consts:
        w1_sb = consts["pool"].tile([P, 2 * CR], F32, tag="w1_sb")
        nc.gpsimd.dma_start(w1_sb[:, 0:CR], w1t_ap[0])
        nc.gpsimd.dma_start(w1_sb[:, CR : 2 * CR], w1t_ap[1])
        w2_sb = consts["pool"].tile([CR, C], F32, tag="w2_sb")
        nc.gpsimd.dma_start(w2_sb[:], w2t_ap[:])
        b1_sb = consts["pool"].tile([CR, 1], F32, tag="b1_sb")
        nc.gpsimd.dma_start(b1_sb[:], b1c_ap[:])
        b2_sb = consts["pool"].tile([P, HALF], F32, tag="b2_sb")
        nc.gpsimd.dma_start(b2_sb[:], b2c_ap[:])
        consts.update(w1_sb=w1_sb, w2_sb=w2_sb, b1_sb=b1_sb, b2_sb=b2_sb)
    w1_sb = consts["w1_sb"]
    w2_sb = consts["w2_sb"]
    b1_sb = consts["b1_sb"]
    b2_sb = consts["b2_sb"]

    gap = small.tile([P, B_PER_CORE * HALF], F32, tag="gap")
    gate = small.tile([P, B_PER_CORE * HALF], F32, tag="gate")

    for b in range(B_PER_CORE):
        # pass 1: load both channel-halves, reduce over spatial
        for h in range(HALF):
            j = b * HALF + h
            t = xs.tile([P, S], F32, tag="xtile")
            nc.sync.dma_start(t[:], x_ap[b, h * P : (h + 1) * P, :])
            nc.vector.reduce_sum(gap[:, j : j + 1], t[:], axis=AX.X)

        # tiny MLP for this sample: gate = sigmoid(w2 @ relu(w1t' @ gapsum + b1) + b2)
        # (1/S mean scale is folded into w1t on the host)
        hp = psum.tile([CR, 1], F32, tag="hp")
        nc.tensor.matmul(
            hp[:], w1_sb[:, 0:CR], gap[:, b * HALF : b * HALF + 1],
            start=True, stop=False,
        )
        nc.tensor.matmul(
            hp[:], w1_sb[:, CR : 2 * CR], gap[:, b * HALF + 1 : b * HALF + 2],
            start=False, stop=True,
        )
        h_sb = small.tile([CR, 1], F32, tag="h_sb")
        nc.scalar.activation(h_sb[:], hp[:], AF.Relu, bias=b1_sb[:, 0:1])
        for h in range(HALF):
            j = b * HALF + h
            apv = psum.tile([P, 1], F32, tag="apv")
            nc.tensor.matmul(
                apv[:], w2_sb[:, h * P : (h + 1) * P], h_sb[:],
                start=True, stop=True,
            )
            nc.scalar.activation(
                gate[:, j : j + 1], apv[:], AF.Sigmoid, bias=b2_sb[:, h : h + 1]
            )

        # pass 2: reload, multiply by per-channel gate (on ACT, keeping DVE
        # free for the reduces), store in half-row chunks so the store of
        # chunk A overlaps the multiply of chunk B
        SH = S // 2
        for h in range(HALF):
            j = b * HALF + h
            t2 = xs.tile([P, S], F32, tag="xtile")
            nc.sync.dma_start(t2[:], x_ap[b, h * P : (h + 1) * P, :])
            for cks, cke in ((0, SH), (SH, S)):
                nc.scalar.mul(t2[:, cks:cke], t2[:, cks:cke], gate[:, j : j + 1])
                nc.scalar.dma_start(
                    out_ap[b, h * P : (h + 1) * P, cks:cke], t2[:, cks:cke]
                )


def build_program(repeats=1):
    nc = bacc.Bacc("TRN2", target_bir_lowering=False, debug=False, num_devices=N_CORES)
    x_ap = nc.dram_tensor("x", [B_PER_CORE, C, S], F32, kind="ExternalInput").ap()
    w1t_ap = nc.dram_tensor("w1t", [2, P, CR], F32, kind="ExternalInput").ap()
    b1c_ap = nc.dram_tensor("b1c", [CR, 1], F32, kind="ExternalInput").ap()
    w2t_ap = nc.dram_tensor("w2t", [CR, C], F32, kind="ExternalInput").ap()
    b2c_ap = nc.dram_tensor("b2c", [P, HALF], F32, kind="ExternalInput").ap()
    out_ap = nc.dram_tensor("out", [B_PER_CORE, C, S], F32, kind="ExternalOutput").ap()
    aps = (x_ap, out_ap, w1t_ap, b1c_ap, w2t_ap, b2c_ap)

    with tile.TileContext(nc) as tc:
        with ExitStack() as ctx:
            consts_pool = ctx.enter_context(tc.tile_pool(name="consts", bufs=1))
            xs = ctx.enter_context(tc.tile_pool(name="xs", bufs=3))
            psum = ctx.enter_context(tc.tile_pool(name="psum", bufs=2, space="PSUM"))
            small = ctx.enter_context(tc.tile_pool(name="small", bufs=2))
            consts = {"pool": consts_pool}
            pools = (consts, xs, psum, small)
            for _ in range(repeats):
                emit_body(tc, aps, pools)
    nc.compile()
    return nc


def prep_inputs(x, w1, b1, w2, b2):
    """Host-side input prep: shard x by batch, fold mean-scale into w1."""
    xs = np.ascontiguousarray(x.reshape(32, C, S))
    w1t = np.ascontiguousarray((w1.T / S).astype(np.float32).reshape(2, P, CR))
    b1c = np.ascontiguousarray(b1.reshape(CR, 1).astype(np.float32))
    w2t = np.ascontiguousarray(w2.T.astype(np.float32))
    b2c = np.ascontiguousarray(b2.reshape(HALF, P).T.astype(np.float32))
    in_maps = []
    for c in range(N_CORES):
        in_maps.append(
            {
                "x": np.ascontiguousarray(xs[c * B_PER_CORE : (c + 1) * B_PER_CORE]),
                "w1t": w1t,
                "b1c": b1c,
                "w2t": w2t,
                "b2c": b2c,
            }
        )
    return in_maps


def kernel(x, w1, b1, w2, b2):
    x = np.asarray(x, dtype=np.float32)
    in_maps = prep_inputs(
        x, np.asarray(w1), np.asarray(b1), np.asarray(w2), np.asarray(b2)
    )
    nc = build_program()
    res = run_bass_kernel_spmd(nc, in_maps, list(range(N_CORES))).results
    out = np.concatenate([res[c]["out"] for c in range(N_CORES)], axis=0)
    return out.reshape(32, C, 112, 112)
